# revision 30
# baseline (speedup 1.0000x reference)
"""Trainium2 Bass kernel for a 3-class per-pixel cross-entropy loss.

reference semantics (numpy):
    p    = softmax(x, axis=1)                    # x [B,3,H,W] f32
    logp = log(clip(p, 1e-8))
    lp_y = logp gathered at class y               # y [B,H,W] int32
    ce   = -weight[y] * lp_y * loss_mask
    out  = sum(ce) / (B*H*W)

Strategy: data-parallel over the batch dim (1 batch element per NeuronCore,
8 cores).  Per pixel with C=3 we compute on-device:
    e_k = exp(x_k)            (ScalarE, bf16 out)
    s   = e0+e1+e2            (VectorE, in place)
    lse = ln(s)               (ScalarE, f32 out)
    x_y = gather over y       (ScalarE copy + VectorE copy_predicated)
    r   = lse - x_y           ( = -log p_y )
    part += min(r, -ln(1e-8)) * mask    (fused VectorE scalar_tensor_tensor)
Per-core output is a [128,1] per-partition partial sum; the host sums the
8x128 partials in float64 and divides by the global pixel count.
"""

import os
import sys

import numpy as np

for _p in ("/opt/trn_rl_repo", os.path.expanduser("~/.axon_site/_ro/trn_rl_repo")):
    if os.path.isdir(_p) and _p not in sys.path:
        sys.path.append(_p)

import concourse.bacc as bacc
import concourse.bass as bass
import concourse.mybir as mybir
import concourse.tile as tile
from concourse.alu_op_type import AluOpType
from concourse.bass_utils import run_bass_kernel_spmd

# Force Exp and Ln to resolve to the one table set containing both
# (natural_log_exp_and_others): the greedy per-function choice alternates
# between exp_and_others and natural_log, costing a ~1.3us ACT_TABLE_LOAD
# per switch.  Set ids are positional, so strip Exp/Ln/Copy from the other
# sets rather than reordering.
_orig_get_activation_tables = bacc.get_activation_tables


def _merged_act_tables(arch):
    tabs = _orig_get_activation_tables(arch)
    AF = mybir.ActivationFunctionType
    combined = [n for n, fns in tabs.items() if AF.Exp in fns and AF.Ln in fns]
    if combined:
        keep = combined[0]
        for n, fns in tabs.items():
            if n != keep:
                fns -= {AF.Exp, AF.Ln, AF.Copy}
    return tabs


bacc.get_activation_tables = _merged_act_tables

B, C, H, W = 8, 3, 1024, 1024
P = 128
N_CORES = 8
FREE = (H * W) // P  # 8192 elements per partition per plane
TILE_F = 1024
# -log(1e-8): upper clamp on -log(p_y), faithful to torch clamp(min=1e-8).log()
CLAMP = 18.420680743952367

F32 = mybir.dt.float32
BF16 = mybir.dt.bfloat16
I32 = mybir.dt.int32


def build(free=FREE, tile_f=TILE_F, weights=None, cdtype=BF16, io_bufs=4, mid_bufs=2):
    """Build the per-core Bass program.

    weights: None when all class weights are 1.0 (the common case; skips the
    per-pixel weight gather), else a tuple of 3 floats baked as immediates.
    """
    assert free % tile_f == 0
    ntiles = free // tile_f
    AF = mybir.ActivationFunctionType

    # Bacc (not raw Bass): its compile pipeline splits multi-sem waits into
    # event semaphores — TRN2 allows at most one sync wait per instruction.
    nc = bacc.Bacc(None)
    x_in = nc.dram_tensor("x", [C, P, free], F32, kind="ExternalInput")
    y_in = nc.dram_tensor("y", [P, free], I32, kind="ExternalInput")
    m_in = nc.dram_tensor("m", [P, free], F32, kind="ExternalInput")
    out = nc.dram_tensor("out", [P, 1], F32, kind="ExternalOutput")

    with tile.TileContext(nc) as tc:
        with (
            tc.tile_pool(name="io", bufs=io_bufs) as io,
            tc.tile_pool(name="mid", bufs=mid_bufs) as mid,
            tc.tile_pool(name="accp", bufs=1) as accp,
        ):
            parts = accp.tile([P, ntiles], F32)
            for i in range(ntiles):
                sl = bass.ts(i, tile_f)
                # one fused DMA for all 3 logit planes: fewer Sync-engine
                # triggers and completion semaphores than 3 separate loads
                xt = io.tile([P, C, tile_f], F32, tag="xt")
                yt = io.tile([P, tile_f], I32, tag="y")
                mt = io.tile([P, tile_f], F32, tag="m")
                nc.sync.dma_start(yt[:], y_in[:, sl])
                if i == 0:
                    # prologue: split the x load into per-plane DMAs on
                    # parallel queues so the first exp starts ~3x earlier
                    for c in range(C):
                        nc.sync.dma_start(xt[:, c, :], x_in[c, :, sl])
                else:
                    nc.sync.dma_start(
                        xt[:], x_in[:, :, sl].rearrange("c p f -> p c f")
                    )
                nc.sync.dma_start(mt[:], m_in[:, sl])
                x0 = xt[:, 0, :]
                x1 = xt[:, 1, :]
                x2 = xt[:, 2, :]

                # e_k = exp(x_k) in bf16; s accumulates in place over et
                et = mid.tile([P, tile_f], cdtype, tag="et")
                e1 = mid.tile([P, tile_f], cdtype, tag="e1")
                e2 = mid.tile([P, tile_f], cdtype, tag="e2")
                nc.scalar.activation(et[:], x0, AF.Exp)
                nc.scalar.activation(e1[:], x1, AF.Exp)
                nc.scalar.activation(e2[:], x2, AF.Exp)

                nc.vector.tensor_tensor(et[:], et[:], e1[:], AluOpType.add)
                nc.vector.tensor_tensor(et[:], et[:], e2[:], AluOpType.add)
                # lse in f32: bf16-rounding lse is the dominant bias term
                # (~1.4e-4 on the final sum); ACT time is dtype-independent.
                lse = mid.tile([P, tile_f], F32, tag="lse")
                nc.scalar.activation(lse[:], et[:], AF.Ln)

                # x_y gather straight from the f32 logit planes
                # (copy_predicated masks must be integer-typed)
                # x_y gather: start from x0, overwrite with x1 where y != 0
                # (y itself is the predicate — nonzero means class 1 or 2),
                # then overwrite with x2 where y == 2.
                is2 = mid.tile([P, tile_f], mybir.dt.uint8, tag="is2")
                nc.vector.tensor_scalar(is2[:], yt[:], 2.0, None, AluOpType.is_equal)
                # e1 is dead after the s-adds: reuse its slots for xy
                xy = mid.tile([P, tile_f], cdtype, tag="e1")
                # base copy on ScalarE (Copy is in the kept table set); ACT has
                # slack once the table thrash is gone and VectorE is critical
                nc.scalar.copy(xy[:], x0)
                nc.vector.copy_predicated(xy[:], yt[:], x1)
                nc.vector.copy_predicated(xy[:], is2[:], x2)

                # r = lse - x_y = -log p_y  (in place over lse)
                r = lse
                nc.vector.tensor_tensor(r[:], lse[:], xy[:], AluOpType.subtract)

                if weights is not None:
                    w0, w1, w2 = (float(v) for v in weights)
                    is1 = mid.tile([P, tile_f], mybir.dt.uint8, tag="is1")
                    nc.vector.tensor_scalar(
                        is1[:], yt[:], 1.0, None, AluOpType.is_equal
                    )
                    wy = mid.tile([P, tile_f], F32, tag="wy")
                    nc.vector.tensor_scalar(
                        wy[:], is1[:], w1 - w0, w0, AluOpType.mult, AluOpType.add
                    )
                    nc.vector.scalar_tensor_tensor(
                        wy[:], is2[:], w2 - w0, wy[:], AluOpType.mult, AluOpType.add
                    )
                    mm = mid.tile([P, tile_f], F32, tag="mm")
                    nc.vector.tensor_tensor(mm[:], mt[:], wy[:], AluOpType.mult)
                else:
                    mm = mt

                # min(r, CLAMP) * mask, row-summed into parts[:, i]
                # (elementwise out written in place over r)
                nc.vector.scalar_tensor_tensor(
                    r[:],
                    r[:],
                    CLAMP,
                    mm[:],
                    AluOpType.min,
                    AluOpType.mult,
                    accum_out=parts[:, i : i + 1],
                )

            acc = accp.tile([P, 1], F32)
            nc.vector.tensor_reduce(acc[:], parts[:], mybir.AxisListType.X, AluOpType.add)
            nc.sync.dma_start(out[:], acc[:])

    nc.finalize()
    return nc


_cache: dict = {}


def _get_nc(weights_key):
    if weights_key not in _cache:
        _cache[weights_key] = build(weights=weights_key)
    return _cache[weights_key]


def _make_in_maps(x, y, loss_mask):
    xs = np.ascontiguousarray(x, dtype=np.float32).reshape(B, C, P, FREE)
    ys = np.ascontiguousarray(y, dtype=np.int32).reshape(B, P, FREE)
    ms = np.ascontiguousarray(loss_mask, dtype=np.float32).reshape(B, P, FREE)
    return [{"x": xs[b], "y": ys[b], "m": ms[b]} for b in range(N_CORES)]


def _ensure_ntff_hook():
    """bass_utils' trace path imports antenv.axon_hooks, which this image
    lacks; synthesize it around the boot script's ctypes NTFF hook."""
    try:
        from antenv.axon_hooks import get_axon_ntff_profile_hook  # noqa: F401

        return
    except ImportError:
        pass
    import types

    hook = None
    try:
        from trn_agent_boot.trn_boot import _ntff_profile_via_ctypes

        so = "/opt/axon/libaxon_pjrt.so"
        if os.path.exists(so):
            hook = _ntff_profile_via_ctypes(so)
    except Exception:
        hook = None
    mod = types.ModuleType("antenv.axon_hooks")
    mod.get_axon_ntff_profile_hook = lambda: hook
    mod.set_axon_ntff_profile_hook = lambda h: None
    sys.modules["antenv.axon_hooks"] = mod
    try:
        import antenv

        antenv.axon_hooks = mod
    except ImportError:
        pass


def run(x, y, weight, loss_mask, trace=False):
    """Run on the 8 NeuronCores; returns (scalar np.float32, exec_time_ns|None)."""
    if trace:
        _ensure_ntff_hook()
    w = np.asarray(weight, dtype=np.float32)
    weights_key = None if np.all(w == 1.0) else tuple(float(v) for v in w)
    nc = _get_nc(weights_key)
    in_maps = _make_in_maps(x, y, loss_mask)
    res = run_bass_kernel_spmd(
        nc, in_maps, core_ids=list(range(N_CORES)), trace=trace
    )
    total = np.float64(0.0)
    for r in res.results:
        total += r["out"].astype(np.float64).sum()
    val = np.float32(total / float(B * H * W))
    return val, res.exec_time_ns


def kernel(x, y, weight, loss_mask):
    val, _ = run(x, y, weight, loss_mask)
    return np.asarray(val, dtype=np.float32)
